# revision 43
# baseline (speedup 1.0000x reference)
"""Trainium2 Bass kernel for nn_Encoder_29661044146233 (gnn_message_passing).

Approach
--------
Both graph blocks are linear per-frame maps on the 88-dim channel vector
(channels = 22 joints x (3 pos + 1 offset)), so they fold into the conv
weights.  The three stride-2 k=4 temporal convs then compose into a single
22-tap stride-8 conv (88 -> 66 channels; the final [..., :3] slice is folded
into the output channels).  Only output frames t=0 and t=255 see boundary
(padding) effects; those two columns are computed with separately probed
15-tap edge weight sets.

Composite weights are obtained on the host by probing the (folded) linear
pipeline with impulses in float64 -- exact to fp32 rounding.  The boundary
correction is a delta: edge-response matrices differ from the interior taps
only for the 3 outermost input frames per side, so t=0/t=255 are fixed up by
adding an 8-matmul [66, 2, 16] correction onto the interior result.

Device kernel (per core, batch 16 of 128):
  - input pre-marshalled on host to channel-major, phase-deinterleaved
    layout [16, 88, 8*258] (one leading + one trailing zero-pad column per
    phase) so every tap's rhs is one contiguous 256-column slice.
  - matmuls in float32r (single-pass fp32, 4x the rate of exact fp32;
    measured ~1.5e-4 rel err on this shape); 22 accumulating matmuls
    (K=88, M=66, N=256) per batch element into PSUM; DVE adds bias while
    copying PSUM -> SBUF, then adds the edge deltas onto cols 0/255.
  - PE pre-warmed with dummy bf16 matmuls so real work runs at 2.4 GHz;
    weights split so the first matmul's operands arrive first; per-batch
    726 KB input DMAs keep the stream dense; stores on the second HWDGE
    ring.  Modeled (TimelineSim) ~49 us/core vs a ~37 us HBM roofline.
"""

import os
import sys

for _p in ("/opt/trn_rl_repo", "/root/.axon_site/_ro/trn_rl_repo"):
    if os.path.isdir(_p) and _p not in sys.path:
        sys.path.append(_p)

import numpy as np

TOPOLOGY = [0, 0, 1, 2, 3, 4, 0, 6, 7, 8, 0, 10, 11, 12, 12, 14, 15, 16, 12, 18, 19, 20]
J = 22
POS, OFF = 3, 1
CIN = 88
COUT = 66
NTAP = 22
NEDGE = 15
B, F, T = 128, 2048, 256
NCORES = 8
BL = B // NCORES          # batch per core
PCOLS = 258               # per-phase columns incl. zero pads
XCOLS = 8 * PCOLS


# ---------------------------------------------------------------------------
# host-side weight composition (float64 impulse probing)
# ---------------------------------------------------------------------------

def _adj():
    a = np.zeros((J, J), np.float64)
    for i, p in enumerate(TOPOLOGY):
        if i:
            a[p, i] = 1.0
    return a


def _conv_np(z, w, b):
    Bn, Fn, C = z.shape
    zp = np.zeros((Bn, Fn + 2, C), z.dtype)
    zp[:, 1:Fn + 1] = z
    Fo = Fn // 2
    out = np.zeros((Bn, Fo, w.shape[0]), z.dtype)
    for k in range(4):
        out += zp[:, k:k + 2 * Fo:2] @ w[:, :, k].T
    return out + b


def _graph_mat(A, n2n_w, n2n_b, e2n_we, e2n_wn, e2n_b,
               n2e_wn, n2e_we, n2e_b, lin_w, lin_b):
    def apply(z):
        sh = z.shape[:-1]
        zz = z.reshape(-1, J, 4)
        node, edge = zz[..., :POS], zz[..., POS:]
        agg_n = np.einsum('ij,bjc->bic', A, node)
        agg_e = np.einsum('ij,bjc->bic', A, edge)
        f1 = agg_n @ n2n_w + n2n_b
        f2 = agg_e @ e2n_we + node @ e2n_wn + e2n_b
        new_edge = (np.einsum('ji,bjc->bic', A, node) @ n2e_wn
                    + edge @ n2e_we + n2e_b)
        h = np.concatenate([f1, f2], axis=-1) @ lin_w + lin_b
        return np.concatenate([h, new_edge], axis=-1).reshape(*sh, 88)

    g = apply(np.zeros((1, 88)))[0]
    G = apply(np.eye(88)) - g
    return G.T, g


def _compose(P):
    A = _adj()
    P64 = {k: np.asarray(v, np.float64) for k, v in P.items()}
    gnames = ('n2n_w', 'n2n_b', 'e2n_we', 'e2n_wn', 'e2n_b',
              'n2e_wn', 'n2e_we', 'n2e_b', 'lin_w', 'lin_b')
    G1, g1 = _graph_mat(A, *[P64['g1_' + s] for s in gnames])
    G2, g2 = _graph_mat(A, *[P64['g2_' + s] for s in gnames])
    keep = np.array([4 * j + c for j in range(J) for c in range(POS)])

    def pipeline(x88):
        y = _conv_np(x88, P64['conv1_w'], P64['conv1_b'])
        y = y @ G1.T + g1
        y = _conv_np(y, P64['conv2_w'], P64['conv2_b'])
        y = y @ G2.T + g2
        y = _conv_np(y, P64['conv3_w'], P64['conv3_b'])
        return y[..., keep]

    Fp = 256
    Tp = Fp // 8
    zb = pipeline(np.zeros((1, Fp, 88)))[0]
    bint, bl, br = zb[Tp // 2], zb[0], zb[Tp - 1]

    mid = Fp // 2
    probes = np.zeros((8 * 88, Fp, 88))
    for r in range(8):
        for ic in range(88):
            probes[r * 88 + ic, mid + r, ic] = 1.0
    resp = pipeline(probes) - zb
    wint = np.zeros((NTAP, COUT, CIN))
    for r in range(8):
        for t in range(Tp):
            m = (mid + r) - 8 * t + 7
            if 0 <= m < NTAP:
                wint[m] = resp[r * 88:(r + 1) * 88, t, :].T

    probes = np.zeros((NEDGE * 88, Fp, 88))
    for f in range(NEDGE):
        for ic in range(88):
            probes[f * 88 + ic, f, ic] = 1.0
    resp = pipeline(probes) - zb
    wl = np.stack([resp[f * 88:(f + 1) * 88, 0, :].T for f in range(NEDGE)])

    probes = np.zeros((NEDGE * 88, Fp, 88))
    for f in range(NEDGE):
        for ic in range(88):
            probes[f * 88 + ic, Fp - NEDGE + f, ic] = 1.0
    resp = pipeline(probes) - zb
    wr = np.stack([resp[f * 88:(f + 1) * 88, Tp - 1, :].T for f in range(NEDGE)])

    return dict(wint=wint, bint=bint, wl=wl, wr=wr, bl=bl, br=br)


def _tap_slice(m):
    # out[t] += W[m] @ x[8t + m - 7]  ->  (phase, col0) in the padded layout
    if m < 7:
        return m + 1, 0
    if m < 15:
        return m - 7, 1
    return m - 15, 2


# ---------------------------------------------------------------------------
# device program (built/compiled once, reused across calls)
# ---------------------------------------------------------------------------

_STATE = {}


def _build_device():
    import concourse.bass as bass  # noqa: F401
    import concourse.tile as tile
    from concourse import bacc, mybir

    f32 = mybir.dt.float32
    # float32r: single-pass fp32 matmul (1 col/cycle vs 4 for exact fp32).
    # Measured on HW for this exact shape: rel err ~1.5e-4 vs float64.
    # Exact fp32 (4x slower on PE) available by flipping this to f32.
    mmdt = mybir.dt.float32r
    nc = bacc.Bacc("TRN2", target_bir_lowering=False, debug=False,
                   num_devices=NCORES)

    # constants in two DMAs:
    #   wb [88, 22*66 + 1]        interior weights + bias col (f32 bits)
    #   we [88, 2*4*(66 + 16)]    edge delta weights + edge inputs
    NDELTA = 4
    CC_W = NTAP * COUT
    CE_W = 2 * NDELTA * COUT
    CE_X = 2 * NDELTA * BL
    wb_d = nc.dram_tensor("wb", [CIN, CC_W + 1], mmdt, kind="ExternalInput")
    we_d = nc.dram_tensor("we", [CIN, CE_W + CE_X], mmdt, kind="ExternalInput")
    xph_d = nc.dram_tensor("xph", [BL, CIN, XCOLS], mmdt, kind="ExternalInput")
    out_d = nc.dram_tensor("out", [COUT, BL, T], f32, kind="ExternalOutput")

    with tile.TileContext(nc) as tc:
        with (
            tc.tile_pool(name="consts", bufs=1) as consts,
            tc.tile_pool(name="xs", bufs=4) as xspool,
            tc.tile_pool(name="xp", bufs=6) as xppool,
            tc.tile_pool(name="ps", bufs=5, space="PSUM") as pspool,
            tc.tile_pool(name="eps", bufs=1, space="PSUM") as epspool,
            tc.tile_pool(name="wrm", bufs=1, space="PSUM") as wrmpool,
            tc.tile_pool(name="ob", bufs=4) as opool,
        ):
            # PE warm-up: dummy matmuls on zeroed scratch, no DMA deps.
            # Runs while the first DMAs stream so real matmuls start at
            # full clock (HAM warm). bf16 on disjoint regions — the
            # vanilla matmul path.
            bf16 = mybir.dt.bfloat16
            scratch = consts.tile([CIN, 512], mmdt)
            nc.vector.memset(scratch[:].bitcast(f32), 0.0)
            s16 = scratch[:].bitcast(bf16)          # [88, 1024] bf16 view
            wps = wrmpool.tile([COUT, 512], f32)
            for _ in range(8):
                nc.tensor.matmul(wps[:], lhsT=s16[:, 0:COUT],
                                 rhs=s16[:, 512:1024], start=True, stop=True)

            def xsingle(b):
                xt = xspool.tile([CIN, XCOLS], mmdt)
                nc.sync.dma_start(out=xt[:], in_=xph_d[b])
                return xt

            # interior weights + bias; one DMA ahead of the input stream
            wb_sb = consts.tile([CIN, CC_W + 1], mmdt)
            nc.sync.dma_start(out=wb_sb[:], in_=wb_d[:])
            w_sb = wb_sb[:, 0:CC_W].rearrange("c (m o) -> c m o", m=NTAP)
            b_sb = wb_sb[:COUT, CC_W:CC_W + 1].bitcast(f32)

            def tap_w(m):
                return w_sb[:, m, :]

            x0 = xsingle(0)
            x1 = xsingle(1)

            we_sb_t = consts.tile([CIN, CE_W + CE_X], mmdt)
            nc.sync.dma_start(out=we_sb_t[:], in_=we_d[:])
            we_sb = we_sb_t[:, 0:CE_W].rearrange(
                "c (s e o) -> c s e o", s=2, e=NDELTA)
            xe_sb = we_sb_t[:, CE_W:CE_W + CE_X].rearrange(
                "c (s e b) -> c s e b", s=2, e=NDELTA)

            # edge delta columns: one accumulation group over both sides
            eps = epspool.tile([COUT, 2, BL], f32)
            for side in range(2):
                for e in range(NDELTA):
                    nc.tensor.matmul(
                        eps[:, side, :],
                        lhsT=we_sb[:, side, e, :],
                        rhs=xe_sb[:, side, e, :],
                        start=(side == 0 and e == 0),
                        stop=(side == 1 and e == NDELTA - 1),
                    )

            def conv_group(ps, xt_ap, nb):
                # xt_ap: [CIN, nb, XCOLS] view; ps: [COUT, nb, T]
                for m in range(NTAP):
                    p, c0 = _tap_slice(m)
                    col = p * PCOLS + c0
                    nc.tensor.matmul(
                        ps[:],
                        lhsT=tap_w(m),
                        rhs=xt_ap[:, :, col:col + 256],
                        start=(m == 0),
                        stop=(m == NTAP - 1),
                    )

            def assemble(ps, b0, nb):
                ob = opool.tile([COUT, nb, T], f32)
                nc.vector.tensor_scalar_add(ob[:], ps[:], b_sb[:])
                for j in range(nb):
                    bi = b0 + j
                    nc.vector.tensor_add(
                        ob[:, j, 0:1], ob[:, j, 0:1], eps[:, 0, bi:bi + 1])
                    nc.vector.tensor_add(
                        ob[:, j, T - 1:T], ob[:, j, T - 1:T], eps[:, 1, bi:bi + 1])
                nc.scalar.dma_start(out=out_d[:, b0:b0 + nb, :], in_=ob[:])

            def single(b, xt):
                ps = pspool.tile([COUT, 1, T], f32)
                conv_group(ps, xt[:].rearrange("c (b x) -> c b x", b=1), 1)
                assemble(ps, b, 1)

            single(0, x0)
            single(1, x1)
            # middle batches as pairs: N=512 matmuls halve the PE
            # instruction count (LDW/NX overhead) at the same stream rate
            for g in range(1, BL // 2 - 1):
                xt = xppool.tile([CIN, 2, XCOLS], mmdt)
                nc.sync.dma_start(
                    out=xt[:],
                    in_=xph_d[2 * g:2 * g + 2].rearrange("b c x -> c b x"))
                ps = pspool.tile([COUT, 2, T], f32)
                conv_group(ps, xt, 2)
                assemble(ps, 2 * g, 2)
            single(BL - 2, xsingle(BL - 2))
            single(BL - 1, xsingle(BL - 1))

    nc.compile()
    return nc


def _get_state():
    if "nc" not in _STATE:
        _STATE["nc"] = _build_device()
    return _STATE["nc"]


# ---------------------------------------------------------------------------
# entry point
# ---------------------------------------------------------------------------

def _kernel_impl(**inputs):
    from concourse.bass_utils import run_bass_kernel_spmd

    P = {k: np.asarray(v) for k, v in inputs.items()}
    inp = P.pop("input").astype(np.float32, copy=False)
    off = P.pop("offset").astype(np.float32, copy=False)

    C = _compose(P)

    x88T = np.ascontiguousarray(
        np.concatenate([inp, off], -1).reshape(B, F, CIN).transpose(0, 2, 1))

    xph = np.zeros((B, CIN, 8, PCOLS), np.float32)
    xph[:, :, :, 1:257] = x88T.reshape(B, CIN, T, 8).transpose(0, 1, 3, 2)
    xph = xph.reshape(B, CIN, XCOLS)

    # edge delta inputs: 3 boundary frames + one e0 (bias) slot per side
    NDELTA = 4
    xedge = np.zeros((B, CIN, 2, NDELTA), np.float32)
    xedge[:, :, 0, :3] = x88T[:, :, :3]
    xedge[:, :, 1, :3] = x88T[:, :, F - 3:]
    xedge[:, 0, :, 3] = 1.0

    wint = np.ascontiguousarray(
        C["wint"].transpose(2, 0, 1)).astype(np.float32)        # [88, 22, 66]
    # delta edge weights: W_edge - W_interior is nonzero only for the
    # 3 outermost frames per side (verified: pad corrections reach <= 9
    # frames in, and only the first/last 3 actually differ)
    dwl = (C["wl"][:3] - C["wint"][7:10]).transpose(2, 0, 1)     # [88, 3, 66]
    dwr = (C["wr"][12:15] - C["wint"][12:15]).transpose(2, 0, 1)
    wedge = np.zeros((CIN, 2, NDELTA, COUT), np.float32)
    wedge[:, 0, :3, :] = dwl
    wedge[:, 1, :3, :] = dwr
    wedge[0, 0, 3, :] = C["bl"] - C["bint"]
    wedge[0, 1, 3, :] = C["br"] - C["bint"]
    bias = np.zeros((CIN, 1), np.float32)
    bias[:COUT, 0] = C["bint"]

    wb = np.concatenate([wint.reshape(CIN, -1), bias], axis=1)
    in_maps = []
    for c in range(NCORES):
        s = slice(c * BL, (c + 1) * BL)
        we = np.concatenate([
            wedge.reshape(CIN, -1),
            np.ascontiguousarray(
                xedge[s].transpose(1, 2, 3, 0)).reshape(CIN, -1),
        ], axis=1)
        in_maps.append({
            "wb": wb,
            "we": we,
            "xph": xph[s],
        })

    nc = _get_state()
    res = run_bass_kernel_spmd(nc, in_maps, core_ids=list(range(NCORES)))

    out = np.empty((B, T, J, POS), np.float32)
    for c in range(NCORES):
        o = res.results[c]["out"]                                # [66, BL, 256]
        out[c * BL:(c + 1) * BL] = o.transpose(1, 2, 0).reshape(BL, T, J, POS)
    return out


def _subproc_main(in_path, out_path):
    with open(in_path, "rb") as f:
        import pickle
        inputs = pickle.load(f)
    np.save(out_path, _kernel_impl(**inputs))


def kernel(**inputs):
    """Entry point. The very first execution of a freshly compiled NEFF
    occasionally kills the device session (NRT_EXEC_UNIT_UNRECOVERABLE);
    a rerun in a fresh process reliably succeeds (the compile cache makes
    it cheap). So: try in-process, fall back to fresh subprocesses."""
    if not _STATE.get("dead"):
        try:
            return _kernel_impl(**inputs)
        except Exception:  # noqa: BLE001
            _STATE["dead"] = True  # this process's device session is gone

    import pickle
    import subprocess
    import tempfile

    kdir = os.path.dirname(os.path.abspath(__file__))
    last_err = None
    for _ in range(3):
        with tempfile.TemporaryDirectory() as td:
            ip = os.path.join(td, "in.pkl")
            op = os.path.join(td, "out.npy")
            with open(ip, "wb") as f:
                pickle.dump({k: np.asarray(v) for k, v in inputs.items()}, f,
                            protocol=4)
            code = (
                "import sys; sys.path.insert(0, {kd!r}); import kernel; "
                "kernel._subproc_main({ip!r}, {op!r})"
            ).format(kd=kdir, ip=ip, op=op)
            r = subprocess.run([sys.executable, "-c", code],
                               capture_output=True, text=True)
            if r.returncode == 0 and os.path.exists(op):
                return np.load(op)
            last_err = r.stderr[-2000:] if r.stderr else f"rc={r.returncode}"
    raise RuntimeError(f"kernel subprocess retries exhausted: {last_err}")


# revision 45
# speedup vs baseline: 1.0339x; 1.0339x over previous
"""Trainium2 Bass kernel for nn_Encoder_29661044146233 (gnn_message_passing).

Approach
--------
Both graph blocks are linear per-frame maps on the 88-dim channel vector
(channels = 22 joints x (3 pos + 1 offset)), so they fold into the conv
weights.  The three stride-2 k=4 temporal convs then compose into a single
22-tap stride-8 conv (88 -> 66 channels; the final [..., :3] slice is folded
into the output channels).  Only output frames t=0 and t=255 see boundary
(padding) effects; those two columns are computed with separately probed
15-tap edge weight sets.

Composite weights are obtained on the host by probing the (folded) linear
pipeline with impulses in float64 -- exact to fp32 rounding.  The boundary
correction is a delta: edge-response matrices differ from the interior taps
only for the 3 outermost input frames per side, so t=0/t=255 are fixed up by
adding an 8-matmul [66, 2, 16] correction onto the interior result.

Device kernel (per core, batch 16 of 128):
  - input pre-marshalled on host to channel-major, phase-deinterleaved
    layout [16, 88, 8*258] (one leading + one trailing zero-pad column per
    phase) so every tap's rhs is one contiguous 256-column slice.
  - matmuls in float32r (single-pass fp32, 4x the rate of exact fp32;
    measured ~1.5e-4 rel err on this shape); 22 accumulating matmuls
    (K=88, M=66, N=256) per batch element into PSUM; DVE adds bias while
    copying PSUM -> SBUF, then adds the edge deltas onto cols 0/255.
  - PE pre-warmed with dummy bf16 matmuls so real work runs at 2.4 GHz;
    weights split so the first matmul's operands arrive first; per-batch
    726 KB input DMAs keep the stream dense; stores on the second HWDGE
    ring.  Modeled (TimelineSim) ~49 us/core vs a ~37 us HBM roofline.
"""

import os
import sys

for _p in ("/opt/trn_rl_repo", "/root/.axon_site/_ro/trn_rl_repo"):
    if os.path.isdir(_p) and _p not in sys.path:
        sys.path.append(_p)

import numpy as np

TOPOLOGY = [0, 0, 1, 2, 3, 4, 0, 6, 7, 8, 0, 10, 11, 12, 12, 14, 15, 16, 12, 18, 19, 20]
J = 22
POS, OFF = 3, 1
CIN = 88
COUT = 66
NTAP = 22
NEDGE = 15
B, F, T = 128, 2048, 256
NCORES = 8
BL = B // NCORES          # batch per core
PCOLS = 258               # per-phase columns incl. zero pads
XCOLS = 8 * PCOLS


# ---------------------------------------------------------------------------
# host-side weight composition (float64 impulse probing)
# ---------------------------------------------------------------------------

def _adj():
    a = np.zeros((J, J), np.float64)
    for i, p in enumerate(TOPOLOGY):
        if i:
            a[p, i] = 1.0
    return a


def _conv_np(z, w, b):
    Bn, Fn, C = z.shape
    zp = np.zeros((Bn, Fn + 2, C), z.dtype)
    zp[:, 1:Fn + 1] = z
    Fo = Fn // 2
    out = np.zeros((Bn, Fo, w.shape[0]), z.dtype)
    for k in range(4):
        out += zp[:, k:k + 2 * Fo:2] @ w[:, :, k].T
    return out + b


def _graph_mat(A, n2n_w, n2n_b, e2n_we, e2n_wn, e2n_b,
               n2e_wn, n2e_we, n2e_b, lin_w, lin_b):
    def apply(z):
        sh = z.shape[:-1]
        zz = z.reshape(-1, J, 4)
        node, edge = zz[..., :POS], zz[..., POS:]
        agg_n = np.einsum('ij,bjc->bic', A, node)
        agg_e = np.einsum('ij,bjc->bic', A, edge)
        f1 = agg_n @ n2n_w + n2n_b
        f2 = agg_e @ e2n_we + node @ e2n_wn + e2n_b
        new_edge = (np.einsum('ji,bjc->bic', A, node) @ n2e_wn
                    + edge @ n2e_we + n2e_b)
        h = np.concatenate([f1, f2], axis=-1) @ lin_w + lin_b
        return np.concatenate([h, new_edge], axis=-1).reshape(*sh, 88)

    g = apply(np.zeros((1, 88)))[0]
    G = apply(np.eye(88)) - g
    return G.T, g


def _compose(P):
    A = _adj()
    P64 = {k: np.asarray(v, np.float64) for k, v in P.items()}
    gnames = ('n2n_w', 'n2n_b', 'e2n_we', 'e2n_wn', 'e2n_b',
              'n2e_wn', 'n2e_we', 'n2e_b', 'lin_w', 'lin_b')
    G1, g1 = _graph_mat(A, *[P64['g1_' + s] for s in gnames])
    G2, g2 = _graph_mat(A, *[P64['g2_' + s] for s in gnames])
    keep = np.array([4 * j + c for j in range(J) for c in range(POS)])

    def pipeline(x88):
        y = _conv_np(x88, P64['conv1_w'], P64['conv1_b'])
        y = y @ G1.T + g1
        y = _conv_np(y, P64['conv2_w'], P64['conv2_b'])
        y = y @ G2.T + g2
        y = _conv_np(y, P64['conv3_w'], P64['conv3_b'])
        return y[..., keep]

    Fp = 256
    Tp = Fp // 8
    zb = pipeline(np.zeros((1, Fp, 88)))[0]
    bint, bl, br = zb[Tp // 2], zb[0], zb[Tp - 1]

    mid = Fp // 2
    probes = np.zeros((8 * 88, Fp, 88))
    for r in range(8):
        for ic in range(88):
            probes[r * 88 + ic, mid + r, ic] = 1.0
    resp = pipeline(probes) - zb
    wint = np.zeros((NTAP, COUT, CIN))
    for r in range(8):
        for t in range(Tp):
            m = (mid + r) - 8 * t + 7
            if 0 <= m < NTAP:
                wint[m] = resp[r * 88:(r + 1) * 88, t, :].T

    probes = np.zeros((NEDGE * 88, Fp, 88))
    for f in range(NEDGE):
        for ic in range(88):
            probes[f * 88 + ic, f, ic] = 1.0
    resp = pipeline(probes) - zb
    wl = np.stack([resp[f * 88:(f + 1) * 88, 0, :].T for f in range(NEDGE)])

    probes = np.zeros((NEDGE * 88, Fp, 88))
    for f in range(NEDGE):
        for ic in range(88):
            probes[f * 88 + ic, Fp - NEDGE + f, ic] = 1.0
    resp = pipeline(probes) - zb
    wr = np.stack([resp[f * 88:(f + 1) * 88, Tp - 1, :].T for f in range(NEDGE)])

    return dict(wint=wint, bint=bint, wl=wl, wr=wr, bl=bl, br=br)


def _tap_slice(m):
    # out[t] += W[m] @ x[8t + m - 7]  ->  (phase, col0) in the padded layout
    if m < 7:
        return m + 1, 0
    if m < 15:
        return m - 7, 1
    return m - 15, 2


# ---------------------------------------------------------------------------
# device program (built/compiled once, reused across calls)
# ---------------------------------------------------------------------------

_STATE = {}


def _build_device():
    import concourse.bass as bass  # noqa: F401
    import concourse.tile as tile
    from concourse import bacc, mybir

    f32 = mybir.dt.float32
    # float32r: single-pass fp32 matmul (1 col/cycle vs 4 for exact fp32).
    # Measured on HW for this exact shape: rel err ~1.5e-4 vs float64.
    # Exact fp32 (4x slower on PE) available by flipping this to f32.
    mmdt = mybir.dt.float32r
    nc = bacc.Bacc("TRN2", target_bir_lowering=False, debug=False,
                   num_devices=NCORES)

    # constants in two DMAs:
    #   wb [88, 22*66 + 1]        interior weights + bias col (f32 bits)
    #   we [88, 2*4*(66 + 16)]    edge delta weights + edge inputs
    NDELTA = 4
    CC_W = NTAP * COUT
    CE_W = 2 * NDELTA * COUT
    CE_X = 2 * NDELTA * BL
    wb_d = nc.dram_tensor("wb", [CIN, CC_W + 1], mmdt, kind="ExternalInput")
    we_d = nc.dram_tensor("we", [CIN, CE_W + CE_X], mmdt, kind="ExternalInput")
    xph_d = nc.dram_tensor("xph", [BL, CIN, XCOLS], mmdt, kind="ExternalInput")
    out_d = nc.dram_tensor("out", [COUT, BL, T], f32, kind="ExternalOutput")

    with tile.TileContext(nc) as tc:
        with (
            tc.tile_pool(name="consts", bufs=1) as consts,
            tc.tile_pool(name="xs", bufs=12) as xspool,
            tc.tile_pool(name="ps", bufs=5, space="PSUM") as pspool,
            tc.tile_pool(name="eps", bufs=1, space="PSUM") as epspool,
            tc.tile_pool(name="wrm", bufs=1, space="PSUM") as wrmpool,
            tc.tile_pool(name="ob", bufs=4) as opool,
        ):
            # PE warm-up: dummy matmuls on zeroed scratch, no DMA deps.
            # Runs while the first DMAs stream so real matmuls start at
            # full clock (HAM warm). bf16 on disjoint regions — the
            # vanilla matmul path.
            bf16 = mybir.dt.bfloat16
            scratch = consts.tile([CIN, 512], mmdt)
            nc.vector.memset(scratch[:].bitcast(f32), 0.0)
            s16 = scratch[:].bitcast(bf16)          # [88, 1024] bf16 view
            wps = wrmpool.tile([COUT, 512], f32)
            for _ in range(8):
                nc.tensor.matmul(wps[:], lhsT=s16[:, 0:COUT],
                                 rhs=s16[:, 512:1024], start=True, stop=True)

            def xsingle(b):
                xt = xspool.tile([CIN, XCOLS], mmdt)
                nc.sync.dma_start(out=xt[:], in_=xph_d[b])
                return xt

            # interior weights + bias; one DMA ahead of the input stream
            wb_sb = consts.tile([CIN, CC_W + 1], mmdt)
            nc.sync.dma_start(out=wb_sb[:], in_=wb_d[:])
            w_sb = wb_sb[:, 0:CC_W].rearrange("c (m o) -> c m o", m=NTAP)
            b_sb = wb_sb[:COUT, CC_W:CC_W + 1].bitcast(f32)

            def tap_w(m):
                return w_sb[:, m, :]

            x0 = xsingle(0)
            x1 = xsingle(1)

            we_sb_t = consts.tile([CIN, CE_W + CE_X], mmdt)
            nc.sync.dma_start(out=we_sb_t[:], in_=we_d[:])
            we_sb = we_sb_t[:, 0:CE_W].rearrange(
                "c (s e o) -> c s e o", s=2, e=NDELTA)
            xe_sb = we_sb_t[:, CE_W:CE_W + CE_X].rearrange(
                "c (s e b) -> c s e b", s=2, e=NDELTA)

            # edge delta columns: one accumulation group over both sides
            eps = epspool.tile([COUT, 2, BL], f32)
            for side in range(2):
                for e in range(NDELTA):
                    nc.tensor.matmul(
                        eps[:, side, :],
                        lhsT=we_sb[:, side, e, :],
                        rhs=xe_sb[:, side, e, :],
                        start=(side == 0 and e == 0),
                        stop=(side == 1 and e == NDELTA - 1),
                    )

            def conv_group(ps, xt_ap, nb):
                # xt_ap: [CIN, nb, XCOLS] view; ps: [COUT, nb, T]
                for m in range(NTAP):
                    p, c0 = _tap_slice(m)
                    col = p * PCOLS + c0
                    nc.tensor.matmul(
                        ps[:],
                        lhsT=tap_w(m),
                        rhs=xt_ap[:, :, col:col + 256],
                        start=(m == 0),
                        stop=(m == NTAP - 1),
                    )

            def assemble(ps, b0, nb):
                ob = opool.tile([COUT, nb, T], f32)
                nc.vector.tensor_scalar_add(ob[:], ps[:], b_sb[:])
                for j in range(nb):
                    bi = b0 + j
                    nc.vector.tensor_add(
                        ob[:, j, 0:1], ob[:, j, 0:1], eps[:, 0, bi:bi + 1])
                    nc.vector.tensor_add(
                        ob[:, j, T - 1:T], ob[:, j, T - 1:T], eps[:, 1, bi:bi + 1])
                nc.scalar.dma_start(out=out_d[:, b0:b0 + nb, :], in_=ob[:])

            def single(b, xt):
                ps = pspool.tile([COUT, 1, T], f32)
                conv_group(ps, xt[:].rearrange("c (b x) -> c b x", b=1), 1)
                assemble(ps, b, 1)

            single(0, x0)
            single(1, x1)
            for b in range(2, BL):
                single(b, xsingle(b))

    nc.compile()
    return nc


def _get_state():
    if "nc" not in _STATE:
        _STATE["nc"] = _build_device()
    return _STATE["nc"]


# ---------------------------------------------------------------------------
# entry point
# ---------------------------------------------------------------------------

def _kernel_impl(**inputs):
    from concourse.bass_utils import run_bass_kernel_spmd

    P = {k: np.asarray(v) for k, v in inputs.items()}
    inp = P.pop("input").astype(np.float32, copy=False)
    off = P.pop("offset").astype(np.float32, copy=False)

    C = _compose(P)

    x88T = np.ascontiguousarray(
        np.concatenate([inp, off], -1).reshape(B, F, CIN).transpose(0, 2, 1))

    xph = np.zeros((B, CIN, 8, PCOLS), np.float32)
    xph[:, :, :, 1:257] = x88T.reshape(B, CIN, T, 8).transpose(0, 1, 3, 2)
    xph = xph.reshape(B, CIN, XCOLS)

    # edge delta inputs: 3 boundary frames + one e0 (bias) slot per side
    NDELTA = 4
    xedge = np.zeros((B, CIN, 2, NDELTA), np.float32)
    xedge[:, :, 0, :3] = x88T[:, :, :3]
    xedge[:, :, 1, :3] = x88T[:, :, F - 3:]
    xedge[:, 0, :, 3] = 1.0

    wint = np.ascontiguousarray(
        C["wint"].transpose(2, 0, 1)).astype(np.float32)        # [88, 22, 66]
    # delta edge weights: W_edge - W_interior is nonzero only for the
    # 3 outermost frames per side (verified: pad corrections reach <= 9
    # frames in, and only the first/last 3 actually differ)
    dwl = (C["wl"][:3] - C["wint"][7:10]).transpose(2, 0, 1)     # [88, 3, 66]
    dwr = (C["wr"][12:15] - C["wint"][12:15]).transpose(2, 0, 1)
    wedge = np.zeros((CIN, 2, NDELTA, COUT), np.float32)
    wedge[:, 0, :3, :] = dwl
    wedge[:, 1, :3, :] = dwr
    wedge[0, 0, 3, :] = C["bl"] - C["bint"]
    wedge[0, 1, 3, :] = C["br"] - C["bint"]
    bias = np.zeros((CIN, 1), np.float32)
    bias[:COUT, 0] = C["bint"]

    wb = np.concatenate([wint.reshape(CIN, -1), bias], axis=1)
    in_maps = []
    for c in range(NCORES):
        s = slice(c * BL, (c + 1) * BL)
        we = np.concatenate([
            wedge.reshape(CIN, -1),
            np.ascontiguousarray(
                xedge[s].transpose(1, 2, 3, 0)).reshape(CIN, -1),
        ], axis=1)
        in_maps.append({
            "wb": wb,
            "we": we,
            "xph": xph[s],
        })

    nc = _get_state()
    res = run_bass_kernel_spmd(nc, in_maps, core_ids=list(range(NCORES)))

    out = np.empty((B, T, J, POS), np.float32)
    for c in range(NCORES):
        o = res.results[c]["out"]                                # [66, BL, 256]
        out[c * BL:(c + 1) * BL] = o.transpose(1, 2, 0).reshape(BL, T, J, POS)
    return out


def _subproc_main(in_path, out_path):
    with open(in_path, "rb") as f:
        import pickle
        inputs = pickle.load(f)
    np.save(out_path, _kernel_impl(**inputs))


def kernel(**inputs):
    """Entry point. The very first execution of a freshly compiled NEFF
    occasionally kills the device session (NRT_EXEC_UNIT_UNRECOVERABLE);
    a rerun in a fresh process reliably succeeds (the compile cache makes
    it cheap). So: try in-process, fall back to fresh subprocesses."""
    if not _STATE.get("dead"):
        try:
            return _kernel_impl(**inputs)
        except Exception:  # noqa: BLE001
            _STATE["dead"] = True  # this process's device session is gone

    import pickle
    import subprocess
    import tempfile

    kdir = os.path.dirname(os.path.abspath(__file__))
    last_err = None
    for _ in range(3):
        with tempfile.TemporaryDirectory() as td:
            ip = os.path.join(td, "in.pkl")
            op = os.path.join(td, "out.npy")
            with open(ip, "wb") as f:
                pickle.dump({k: np.asarray(v) for k, v in inputs.items()}, f,
                            protocol=4)
            code = (
                "import sys; sys.path.insert(0, {kd!r}); import kernel; "
                "kernel._subproc_main({ip!r}, {op!r})"
            ).format(kd=kdir, ip=ip, op=op)
            r = subprocess.run([sys.executable, "-c", code],
                               capture_output=True, text=True)
            if r.returncode == 0 and os.path.exists(op):
                return np.load(op)
            last_err = r.stderr[-2000:] if r.stderr else f"rc={r.returncode}"
    raise RuntimeError(f"kernel subprocess retries exhausted: {last_err}")
